# revision 14
# baseline (speedup 1.0000x reference)
"""GAT 2-layer classifier on 8 Trainium2 NeuronCores.

Strategy (edge partition by dst, node-major processing):
 - Nodes are split 6250/core by dst range; each core processes exactly the
   edges landing in its dst range, so all segment sums are core-local.
 - Within a core, nodes are permuted by in-shard degree (descending) and
   grouped 128/group; each group g has a compile-time slot count K[g]
   (max degree in the group across all cores). Edge slots are padded with a
   sentinel "zero row" whose el entry is -1e30 -> ex = exp(lrelu(...)) = 0.
 - Per layer a "record table" [h | el | er] (136 f32 = 544B rows) is
   computed on every core via bf16 PE matmuls: h_ext = X @ [W | W@a_l | W@a_r].
 - Edge gathers use gpsimd indirect DMA (int32 row indices, one [128,1]
   offset call per slot column — the only reliable form on this target)
   into SBUF tiles [128 nodes, K, 136]; er is per-partition (free),
   el from the gathered record; messages are scaled by ex on DVE and
   reduced over K; division by the ex-sum, bias and relu close the layer.
 - One AllGather of the layer-1 activations (feature-major) lets every core
   build the full layer-2 table locally. Logits are computed per group via
   PE transpose + matmul with Wc; the host undoes the node permutation.
"""

import os
import numpy as np

N = 50000
E = 800000
IN_DIM = 128
H = 4
D = 32
HD = H * D  # 128
NCLS = 10
SLOPE = 0.2
CORES = 8
NPC = N // CORES            # 6250 real nodes per core
G = 49                      # groups of 128 node-positions per core
NPC_PAD = G * 128           # 6272 node positions per core
TROWS = CORES * NPC_PAD + 1  # 50177 table rows (last = sentinel zero row)
ZROW = CORES * NPC_PAD       # 50176
EXT = 136                   # record floats: [h(128) | el(4) | er(4)] = 544B
EL_SENT = -1.0e30

LAST_RESULTS = None  # populated when GAT_TRACE env var is set


# ----------------------------------------------------------------------------
# host-side preprocessing (pure index manipulation + weight packing)
# ----------------------------------------------------------------------------

def _prep_graph(src, dst):
    """Shard edges by dst, permute nodes by degree, build gather indices.

    Returns (Ks, gidx_list, oidx_list, orders) where
      Ks:   [G] slot counts per group (uniform across cores)
      gidx: per core int32 [128, sum(Ks)] gather row indices (ZROW = pad)
      oidx: per core int32 [128, G] own-row indices
      orders: per core int32 [NPC] position -> original local node id
    """
    core_of_edge = dst // NPC

    per_core = []
    deg_sorted_all = np.zeros((CORES, NPC_PAD), np.int64)
    for c in range(CORES):
        m = core_of_edge == c
        e_src = src[m].astype(np.int64)
        e_dstl = (dst[m] - c * NPC).astype(np.int64)
        deg = np.bincount(e_dstl, minlength=NPC)
        order = np.argsort(-deg, kind="stable").astype(np.int64)
        inv = np.empty(NPC, np.int64)
        inv[order] = np.arange(NPC)
        deg_sorted_all[c, :NPC] = deg[order]
        per_core.append((e_src, e_dstl, order, inv))

    # group slot schedule: max degree within each 128-node group, across cores
    Ks = deg_sorted_all.reshape(CORES, G, 128).max(axis=2).max(axis=0)
    Ks = np.maximum(Ks, 1).astype(np.int64)
    SK = int(Ks.sum())
    colbase = np.concatenate([[0], np.cumsum(Ks)])  # per-group column offset

    # global storage index of original node n (uniform across cores)
    sidx = np.empty(N, np.int64)
    for c in range(CORES):
        _, _, order, inv = per_core[c]
        sidx[c * NPC:(c + 1) * NPC] = c * NPC_PAD + inv

    gidx_list, oidx_list, orders = [], [], []
    pp, gg = np.meshgrid(np.arange(128), np.arange(G), indexing="ij")
    for c in range(CORES):
        e_src, e_dstl, order, inv = per_core[c]
        pos = inv[e_dstl]                       # node position of each edge
        s = np.argsort(pos, kind="stable")
        spos = pos[s]
        ssrc = sidx[e_src[s]]                   # already table-row indices
        cnt = np.bincount(spos, minlength=NPC_PAD)
        off = np.concatenate([[0], np.cumsum(cnt)])
        k = np.arange(len(spos)) - off[spos]    # slot index within node
        g = spos // 128
        p = spos % 128
        col = colbase[g] + k
        gidx = np.full((128, SK), ZROW, np.int64)
        gidx[p, col] = ssrc
        gidx_list.append(gidx.astype(np.int32))
        oidx_list.append((c * NPC_PAD + gg * 128 + pp).astype(np.int32))
        orders.append(order.astype(np.int64))
    return Ks, gidx_list, oidx_list, orders


def _pack_wext(W, al, ar):
    """[K, EXT] = [W | W@a_l per head | W@a_r per head]."""
    Kin = W.shape[0]
    out = np.zeros((Kin, EXT), np.float32)
    out[:, :HD] = W
    Wh = W.reshape(Kin, H, D)
    out[:, HD:HD + H] = np.einsum("khd,hd->kh", Wh, al)
    out[:, HD + H:HD + 2 * H] = np.einsum("khd,hd->kh", Wh, ar)
    return out


def _build_xt(features, orders):
    """Feature-major, node-permuted input [128, CORES*NPC_PAD]."""
    xt = np.zeros((IN_DIM, CORES * NPC_PAD), np.float32)
    for c in range(CORES):
        cols = features[c * NPC + orders[c]].T  # [128, NPC]
        xt[:, c * NPC_PAD:c * NPC_PAD + NPC] = cols
    return xt


# ----------------------------------------------------------------------------
# bass program
# ----------------------------------------------------------------------------

def _build_program(Ks):
    import concourse.bacc as bacc
    import concourse.bass as bass
    import concourse.mybir as mybir
    import concourse.tile as tile
    from concourse.bass import IndirectOffsetOnAxis
    from concourse.masks import make_identity

    dt = mybir.dt
    SK = int(sum(Ks))
    colbase = np.concatenate([[0], np.cumsum(Ks)]).astype(int)

    nc = bacc.Bacc(None, target_bir_lowering=False, debug=False)

    # kernel I/O
    xt = nc.dram_tensor("xt", [IN_DIM, CORES * NPC_PAD], dt.bfloat16,
                        kind="ExternalInput")
    w1e = nc.dram_tensor("w1e", [IN_DIM, EXT], dt.bfloat16, kind="ExternalInput")
    w2e = nc.dram_tensor("w2e", [HD, EXT], dt.bfloat16, kind="ExternalInput")
    wc = nc.dram_tensor("wc", [HD, NCLS], dt.bfloat16, kind="ExternalInput")
    b1r = nc.dram_tensor("b1r", [128, HD], dt.float32, kind="ExternalInput")
    b2r = nc.dram_tensor("b2r", [128, HD], dt.float32, kind="ExternalInput")
    bcr = nc.dram_tensor("bcr", [128, NCLS], dt.float32, kind="ExternalInput")
    zrow = nc.dram_tensor("zrow", [1, EXT], dt.float32, kind="ExternalInput")
    gidx = nc.dram_tensor("gidx", [128, SK], dt.int32, kind="ExternalInput")
    oidx = nc.dram_tensor("oidx", [128, G], dt.int32, kind="ExternalInput")
    logits = nc.dram_tensor("logits", [NPC_PAD, NCLS], dt.float32,
                            kind="ExternalOutput")

    # internal DRAM
    debug = bool(os.environ.get("GAT_DEBUG"))
    tkind = {"kind": "ExternalOutput"} if debug else {}
    t1 = nc.dram_tensor("t1", [TROWS, EXT], dt.float32, **tkind)
    t2 = nc.dram_tensor("t2", [TROWS, EXT], dt.float32, **tkind)
    a1t_loc = nc.dram_tensor("a1t_loc", [128, NPC_PAD], dt.bfloat16)
    if debug:
        a1t_dbg = nc.dram_tensor("a1t_dbg", [128, NPC_PAD], dt.bfloat16,
                                 kind="ExternalOutput")
    a1t_full = nc.dram_tensor("a1t_full", [CORES * 128, NPC_PAD], dt.bfloat16,
                              addr_space="Shared")

    TCH = 3  # node tiles batched per PSUM flush in table phases

    with tile.TileContext(nc) as tc:
        with tc.tile_pool(name="consts", bufs=1) as cp:
            w1e_sb = cp.tile([IN_DIM, EXT], dt.bfloat16)
            w2e_sb = cp.tile([HD, EXT], dt.bfloat16)
            wc_sb = cp.tile([HD, NCLS], dt.bfloat16)
            b1r_sb = cp.tile([128, HD], dt.float32)
            b2r_sb = cp.tile([128, HD], dt.float32)
            bcr_sb = cp.tile([128, NCLS], dt.float32)
            zrow_sb = cp.tile([1, EXT], dt.float32)
            ident = cp.tile([128, 128], dt.float32)
            gidx_sb = cp.tile([128, SK], dt.int32)
            oidx_sb = cp.tile([128, G], dt.int32)

            nc.sync.dma_start(w1e_sb[:], w1e[:, :])
            nc.sync.dma_start(w2e_sb[:], w2e[:, :])
            nc.sync.dma_start(wc_sb[:], wc[:, :])
            nc.sync.dma_start(b1r_sb[:], b1r[:, :])
            nc.sync.dma_start(b2r_sb[:], b2r[:, :])
            nc.sync.dma_start(bcr_sb[:], bcr[:, :])
            nc.sync.dma_start(zrow_sb[:], zrow[:, :])
            nc.sync.dma_start(gidx_sb[:], gidx[:, :])
            nc.sync.dma_start(oidx_sb[:], oidx[:, :])
            make_identity(nc, ident[:])

            # sentinel rows of both tables
            nc.sync.dma_start(t1[ZROW:ZROW + 1, :], zrow_sb[:])
            nc.sync.dma_start(t2[ZROW:ZROW + 1, :], zrow_sb[:])

            def build_table(tbl, w_sb, src_cols):
                """tbl rows <- (columns of src_cols).T @ w_sb, per (s, tt)."""
                with tc.tile_pool(name="tp", bufs=3) as tp, \
                     tc.tile_pool(name="tps", bufs=3, space="PSUM") as tps:
                    for s in range(CORES):
                        for t0 in range(0, G, TCH):
                            nt = min(TCH, G - t0)
                            lh = tp.tile([128, TCH * 128], dt.bfloat16, tag="lh")
                            nc.sync.dma_start(
                                lh[:, :nt * 128],
                                src_cols(s, t0 * 128, nt * 128))
                            ps = tps.tile([128, TCH * EXT], dt.float32, tag="ps")
                            for j in range(nt):
                                nc.tensor.matmul(
                                    ps[:, j * EXT:(j + 1) * EXT],
                                    lhsT=lh[:, j * 128:(j + 1) * 128],
                                    rhs=w_sb[:],
                                    start=True, stop=True)
                            ob = tp.tile([128, TCH * EXT], dt.float32, tag="ob")
                            eng = nc.vector if (t0 // TCH) % 2 == 0 else nc.scalar
                            if eng is nc.vector:
                                eng.tensor_copy(ob[:, :nt * EXT], ps[:, :nt * EXT])
                            else:
                                eng.activation(ob[:, :nt * EXT], ps[:, :nt * EXT],
                                               mybir.ActivationFunctionType.Copy)
                            r0 = s * NPC_PAD + t0 * 128
                            dst = tbl[r0:r0 + nt * 128, :].rearrange(
                                "(j p) f -> p j f", p=128)
                            nc.sync.dma_start(
                                dst, ob[:, :nt * EXT].rearrange(
                                    "p (j f) -> p j f", f=EXT))

            def xt_cols(s, c0, w):
                return xt[:, s * NPC_PAD + c0:s * NPC_PAD + c0 + w]

            def a1_cols(s, c0, w):
                return a1t_full[s * 128:(s + 1) * 128, c0:c0 + w]

            def edge_layer(tbl, brep_sb, last):
                own_pool = tc.tile_pool(name="own", bufs=1)
                er_loc = cp.tile([128, G * H], dt.float32,
                                 tag="er1" if not last else "er2")
                with own_pool as op:
                    own = op.tile([128, G * EXT], dt.float32)
                    for g in range(G):
                        nc.gpsimd.indirect_dma_start(
                            out=own[:, g * EXT:(g + 1) * EXT],
                            out_offset=None,
                            in_=tbl[:, :],
                            in_offset=IndirectOffsetOnAxis(
                                ap=oidx_sb[:, g:g + 1], axis=0))
                    nc.vector.tensor_copy(
                        er_loc[:].rearrange("p (g h) -> p g h", h=H),
                        own[:].rearrange("p (g f) -> p g f", f=EXT)
                           [:, :, HD + H:HD + 2 * H])

                with tc.tile_pool(name="rec", bufs=3) as rp, \
                     tc.tile_pool(name="sc", bufs=3) as scp, \
                     tc.tile_pool(name="sm", bufs=6) as smp, \
                     tc.tile_pool(name="lps", bufs=2, space="PSUM") as lps, \
                     tc.tile_pool(name="cps", bufs=2, space="PSUM") as cps:
                    KMAX = int(max(Ks))
                    for g in range(G):
                        K = int(Ks[g])
                        rec = rp.tile([128, KMAX * EXT], dt.float32, tag="rec")
                        recv = rec[:, :K * EXT].rearrange(
                            "p (k f) -> p k f", f=EXT)
                        for k in range(K):
                            nc.gpsimd.indirect_dma_start(
                                out=rec[:, k * EXT:(k + 1) * EXT],
                                out_offset=None,
                                in_=tbl[:, :],
                                in_offset=IndirectOffsetOnAxis(
                                    ap=gidx_sb[:, colbase[g] + k:colbase[g] + k + 1],
                                    axis=0))
                        # e = el + er ; ex = exp(lrelu(e))
                        epre = smp.tile([128, KMAX * H], dt.float32, tag="epre")
                        er_b = er_loc[:].rearrange(
                            "p (g h) -> p g h", h=H)[:, g:g + 1, :] \
                            .broadcast_to([128, K, H])
                        nc.vector.tensor_tensor(
                            out=epre[:, :K * H].rearrange("p (k h) -> p k h", h=H),
                            in0=recv[:, :, HD:HD + H],
                            in1=er_b,
                            op=mybir.AluOpType.add)
                        # leaky-relu = max(x, 0.2*x) on DVE (ACT Lrelu ignores
                        # the alpha operand — measured slope 0.01)
                        lrs = smp.tile([128, KMAX * H], dt.float32, tag="lrs")
                        nc.vector.tensor_scalar_mul(lrs[:, :K * H],
                                                    epre[:, :K * H], SLOPE)
                        lr = smp.tile([128, KMAX * H], dt.float32, tag="lr")
                        nc.vector.tensor_tensor(out=lr[:, :K * H],
                                                in0=lrs[:, :K * H],
                                                in1=epre[:, :K * H],
                                                op=mybir.AluOpType.max)
                        ex = smp.tile([128, KMAX * H], dt.float32, tag="ex")
                        nc.scalar.activation(ex[:, :K * H], lr[:, :K * H],
                                             mybir.ActivationFunctionType.Exp)
                        # denom and reciprocal
                        den = smp.tile([128, H], dt.float32, tag="den")
                        nc.vector.tensor_reduce(
                            den[:], ex[:, :K * H].rearrange("p (k h) -> p h k", h=H),
                            axis=mybir.AxisListType.X, op=mybir.AluOpType.add)
                        rcp = smp.tile([128, H], dt.float32, tag="rcp")
                        nc.vector.tensor_scalar_max(den[:], den[:], 1e-20)
                        nc.vector.reciprocal(rcp[:], den[:])
                        # scaled messages and K-reduction
                        sc = scp.tile([128, KMAX * HD], dt.float32, tag="sc")
                        nc.vector.tensor_tensor(
                            out=sc[:, :K * HD].rearrange(
                                "p (k h d) -> p k h d", h=H, d=D),
                            in0=recv[:, :, :HD].rearrange(
                                "p k (h d) -> p k h d", d=D),
                            in1=ex[:, :K * H].rearrange(
                                "p (k h o) -> p k h o", h=H, o=1)
                                .broadcast_to([128, K, H, D]),
                            op=mybir.AluOpType.mult)
                        sums = smp.tile([128, HD], dt.float32, tag="sums")
                        nc.vector.tensor_reduce(
                            sums[:], sc[:, :K * HD].rearrange(
                                "p (k f) -> p f k", f=HD),
                            axis=mybir.AxisListType.X, op=mybir.AluOpType.add)
                        # out = relu(sums / den + b)
                        outn = smp.tile([128, HD], dt.float32, tag="outn")
                        nc.vector.tensor_tensor(
                            out=outn[:].rearrange("p (h d) -> p h d", d=D),
                            in0=sums[:].rearrange("p (h d) -> p h d", d=D),
                            in1=rcp[:].rearrange("p (h o) -> p h o", o=1)
                                .broadcast_to([128, H, D]),
                            op=mybir.AluOpType.mult)
                        outb = smp.tile([128, HD], dt.float32, tag="outb")
                        nc.vector.tensor_tensor(out=outb[:], in0=outn[:],
                                                in1=brep_sb[:],
                                                op=mybir.AluOpType.add)
                        outa = smp.tile([128, HD], dt.float32, tag="outa")
                        nc.vector.tensor_scalar_max(outa[:], outb[:], 0.0)
                        # transpose to feature-major
                        pst = lps.tile([128, 128], dt.float32, tag="pst")
                        nc.tensor.transpose(pst[:], outa[:], ident[:])
                        at = smp.tile([128, 128], dt.bfloat16, tag="at")
                        if g % 2 == 0:
                            nc.vector.tensor_copy(at[:], pst[:])
                        else:
                            nc.scalar.activation(
                                at[:], pst[:], mybir.ActivationFunctionType.Copy)
                        if not last:
                            nc.sync.dma_start(
                                a1t_loc[:, g * 128:(g + 1) * 128], at[:])
                        else:
                            lgp = cps.tile([128, NCLS], dt.float32, tag="lgp")
                            nc.tensor.matmul(lgp[:], lhsT=at[:], rhs=wc_sb[:],
                                             start=True, stop=True)
                            lgs = smp.tile([128, NCLS], dt.float32, tag="lgs")
                            nc.vector.tensor_tensor(out=lgs[:], in0=lgp[:],
                                                    in1=bcr_sb[:],
                                                    op=mybir.AluOpType.add)
                            nc.sync.dma_start(
                                logits[g * 128:(g + 1) * 128, :], lgs[:])

            build_table(t1, w1e_sb, xt_cols)
            edge_layer(t1, b1r_sb, last=False)
            if debug:
                nc.sync.dma_start(a1t_dbg[:, :], a1t_loc[:, :])
            nc.gpsimd.collective_compute(
                "AllGather",
                mybir.AluOpType.bypass,
                replica_groups=[list(range(CORES))],
                ins=[a1t_loc[:, :]],
                outs=[a1t_full[:, :]],
            )
            build_table(t2, w2e_sb, a1_cols)
            edge_layer(t2, b2r_sb, last=True)

    nc.finalize()
    return nc


# ----------------------------------------------------------------------------
# entry point
# ----------------------------------------------------------------------------

def _install_trace_shim():
    """Recreate the missing antenv.axon_hooks NTFF-profile hook via ctypes
    (mirrors trn_agent_boot.trn_boot._ntff_profile_via_ctypes)."""
    import sys
    import types
    if "antenv.axon_hooks" in sys.modules:
        return
    mod = types.ModuleType("antenv.axon_hooks")
    _h = [None]
    mod.set_axon_ntff_profile_hook = lambda h: _h.__setitem__(0, h)
    mod.get_axon_ntff_profile_hook = lambda: _h[0]
    sys.modules["antenv.axon_hooks"] = mod
    try:
        import antenv
        antenv.axon_hooks = mod
    except ImportError:
        pass
    try:
        from trn_agent_boot.trn_boot import _ntff_profile_via_ctypes
        hook = _ntff_profile_via_ctypes("/opt/axon/libaxon_pjrt.so")
        if hook is not None:
            mod.set_axon_ntff_profile_hook(hook)
    except Exception as e:  # noqa: BLE001
        print(f"trace shim unavailable: {e}")


def kernel(features, src, dst, W1, al1, ar1, b1, W2, al2, ar2, b2, Wc, bc):
    global LAST_RESULTS
    if os.environ.get("GAT_TRACE"):
        _install_trace_shim()
    from concourse.bass_utils import run_bass_kernel_spmd

    features = np.asarray(features, np.float32)
    src = np.asarray(src)
    dst = np.asarray(dst)

    Ks, gidx_list, oidx_list, orders = _prep_graph(src, dst)
    import ml_dtypes
    xt_np = _build_xt(features, orders).astype(ml_dtypes.bfloat16)
    w1e_np = _pack_wext(np.asarray(W1, np.float32), np.asarray(al1, np.float32),
                        np.asarray(ar1, np.float32)).astype(ml_dtypes.bfloat16)
    w2e_np = _pack_wext(np.asarray(W2, np.float32), np.asarray(al2, np.float32),
                        np.asarray(ar2, np.float32)).astype(ml_dtypes.bfloat16)
    wc_np = np.asarray(Wc, np.float32).astype(ml_dtypes.bfloat16)
    b1r_np = np.broadcast_to(np.asarray(b1, np.float32), (128, HD)).copy()
    b2r_np = np.broadcast_to(np.asarray(b2, np.float32), (128, HD)).copy()
    bcr_np = np.broadcast_to(np.asarray(bc, np.float32), (128, NCLS)).copy()
    zrow_np = np.zeros((1, EXT), np.float32)
    zrow_np[0, HD:HD + H] = EL_SENT

    nc = _build_program(Ks)

    in_maps = []
    for c in range(CORES):
        in_maps.append({
            "xt": xt_np, "w1e": w1e_np, "w2e": w2e_np, "wc": wc_np,
            "b1r": b1r_np, "b2r": b2r_np, "bcr": bcr_np, "zrow": zrow_np,
            "gidx": gidx_list[c], "oidx": oidx_list[c],
        })

    res = run_bass_kernel_spmd(
        nc, in_maps, core_ids=list(range(CORES)),
        trace=bool(os.environ.get("GAT_TRACE")),
    )
    LAST_RESULTS = res

    out = np.empty((N, NCLS), np.float32)
    for c in range(CORES):
        lg = res.results[c]["logits"]
        out[c * NPC + orders[c]] = lg[:NPC]
    return out
